# revision 8
# baseline (speedup 1.0000x reference)
"""Trainium2 Bass kernel for C = triu(triu(A) @ triu(B)), N=4096, fp32.

Math: the product of upper-triangular matrices is upper-triangular, so with
host-side triu masking of A and B the kernel output needs no masking: for an
output tile (m, n) (128x128 tile indices), the contraction over k only gets
contributions from k in [m, n]; tiles below the diagonal are exactly zero.

Sharding (8 cores, SPMD, one NEFF): block-cyclic rows. Core j owns the four
128-row tiles {j, 8+j, 16+j, 24+j} of A and C (512 rows per core); B
(triu-masked) is replicated. All cores run the identical program; where the
program's k-range extends past a core's actual triangle the masked A columns
are zero, so the extra matmuls accumulate zeros and stay correct. This makes
the per-core instruction streams (and hence runtimes) identical by
construction - no load imbalance.

Per-core program: the transposed A shard (lhsT layout, [128, 32, 512]) stays
SBUF-resident; B is streamed tile-by-tile ([128, 512], each tile touched
exactly once); C accumulates in PSUM banks (up to 4 live), is copied out via
VectorE and DMAed to DRAM.

Loop structure: for each 512-wide column super-block s (8 of them), for each
k-tile <= 4s+3, load B[k, s-block] once and matmul it against the A tiles of
every owned row-slot t with 8t <= k, accumulating into psum[t].
"""

import os
import sys

for _p in ("/opt/trn_rl_repo", "/root/.axon_site/_ro/trn_rl_repo"):
    if _p not in sys.path:
        sys.path.insert(0, _p)

import numpy as np

N = 4096
P = 128
NCORES = 8
NSLOT = 4  # row-tiles per core
SW = 512  # n super-block width
NS = N // SW  # 8 supers
KT = N // P  # 32 k-tiles

# matmul input dtype: "f32" (exact, 4 cyc/row), "f32r" (tf32-like, 1 cyc/row
# at free dim >= 256), "bf16" (1 cyc/row, half the DMA traffic)
MM_DTYPE = os.environ.get("MM_DTYPE", "f32")

_cache = {}


def _build(dt_mode):
    import concourse.bacc as bacc
    import concourse.mybir as mybir
    import concourse.tile as tile

    D = {
        "f32": mybir.dt.float32,
        "f32r": mybir.dt.float32r,
        "bf16": mybir.dt.bfloat16,
    }[dt_mode]

    nc = bacc.Bacc(None, target_bir_lowering=False)
    AT = nc.dram_tensor("AT", [P, KT, NSLOT * P], D, kind="ExternalInput")
    # B packed per n-super: B_packed[s, p, ko, w] = triu(B)[128*ko + p, 512*s + w]
    # so a k-chunk load is per-partition contiguous (KCHUNK*512 elements).
    Bm = nc.dram_tensor("B", [NS, P, KT, SW], D, kind="ExternalInput")
    Cm = nc.dram_tensor("C", [NSLOT * P, N], mybir.dt.float32, kind="ExternalOutput")

    KCHUNK = 4
    b_bufs = 12 if dt_mode == "bf16" else 6

    with tile.TileContext(nc) as tc:
        with (
            tc.tile_pool(name="a", bufs=4) as apool,
            tc.tile_pool(name="b", bufs=b_bufs) as bpool,
            tc.tile_pool(name="o", bufs=4) as opool,
            tc.tile_pool(name="ps", bufs=8, space="PSUM") as pspool,
        ):
            # A shard resident in 4 independent tiles so early matmuls only
            # wait on the first chunk
            a_tiles = []
            for g in range(4):
                ag = apool.tile([P, 8, NSLOT * P], D, tag=f"a{g}", name="ag")
                nc.sync.dma_start(ag[:], AT[:, 8 * g : 8 * (g + 1), :])
                a_tiles.append(ag)

            for s in range(NS):
                kmax = 4 * s + 3
                nslots = kmax // 8 + 1
                psums = [
                    pspool.tile([P, SW], mybir.dt.float32, tag="ps", name="ps")
                    for _ in range(nslots)
                ]
                for kc in range(0, kmax + 1, KCHUNK):
                    cnt = min(KCHUNK, kmax + 1 - kc)
                    bt = bpool.tile([P, KCHUNK, SW], D, tag="b", name="bt")
                    nc.sync.dma_start(bt[:, :cnt, :], Bm[s, :, kc : kc + cnt, :])
                    for k in range(kc, kc + cnt):
                        # columns left of 128*(k - 4s) are k < n-tile regions
                        # where triu(B) is zero; skip them
                        w0 = max(0, P * (k - 4 * s))
                        for t in range(k // 8 + 1):
                            nc.tensor.matmul(
                                psums[t][:, w0:SW],
                                a_tiles[k // 8][:, k % 8, P * t : P * (t + 1)],
                                bt[:, k - kc, w0:SW],
                                start=(k == 8 * t),
                                stop=(k == kmax),
                            )
                for t in range(nslots):
                    ot = opool.tile([P, SW], mybir.dt.float32, tag="o", name="ot")
                    nc.vector.tensor_copy(ot[:], psums[t][:])
                    nc.sync.dma_start(
                        Cm[P * t : P * (t + 1), SW * s : SW * (s + 1)], ot[:]
                    )
    nc.compile()
    return nc


def _get_nc():
    if MM_DTYPE not in _cache:
        _cache[MM_DTYPE] = _build(MM_DTYPE)
    return _cache[MM_DTYPE]


def _np_dtype():
    if MM_DTYPE == "bf16":
        import ml_dtypes

        return np.dtype(ml_dtypes.bfloat16)
    return np.dtype(np.float32)


def _make_in_maps(A, B):
    A = np.asarray(A, dtype=np.float32)
    B = np.asarray(B, dtype=np.float32)
    Au = np.triu(A)
    Bu = np.triu(B)

    npdt = _np_dtype()
    # pack: B_packed[s, p, ko, w] = Bu[128*ko + p, 512*s + w]
    Bu_c = np.ascontiguousarray(
        Bu.reshape(KT, P, NS, SW).transpose(2, 1, 0, 3)
    )
    if npdt != np.float32:
        Bu_c = Bu_c.astype(npdt)

    in_maps = []
    for j in range(NCORES):
        rows = np.concatenate(
            [
                np.arange(P * (NCORES * t + j), P * (NCORES * t + j) + P)
                for t in range(NSLOT)
            ]
        )
        A_loc = Au[rows, :]  # [512, 4096]
        # lhsT layout [p, ko, ml]: element = A_loc[ml, ko*128 + p]
        ATd = np.ascontiguousarray(
            A_loc.reshape(NSLOT * P, KT, P).transpose(2, 1, 0)
        )
        if npdt != np.float32:
            ATd = ATd.astype(npdt)
        in_maps.append({"AT": ATd, "B": Bu_c})
    return in_maps


def kernel(A, B):
    from concourse.bass_utils import run_bass_kernel_spmd

    in_maps = _make_in_maps(A, B)
    nc = _get_nc()
    res = run_bass_kernel_spmd(nc, in_maps, core_ids=list(range(NCORES)))

    C = np.zeros((N, N), dtype=np.float32)
    for j in range(NCORES):
        Cj = res.results[j]["C"]
        for t in range(NSLOT):
            m = NCORES * t + j
            C[P * m : P * (m + 1), :] = Cj[P * t : P * (t + 1), :]
    return C


# revision 10
# speedup vs baseline: 1.0874x; 1.0874x over previous
"""Trainium2 Bass kernel for C = triu(triu(A) @ triu(B)), N=4096, fp32.

Math: the product of upper-triangular matrices is upper-triangular, so with
host-side triu masking of A and B the kernel output needs no masking: for an
output tile (m, n) (128x128 tile indices), the contraction over k only gets
contributions from k in [m, n]; tiles below the diagonal are exactly zero.

Sharding (8 cores, SPMD, one NEFF): block-cyclic rows. Core j owns the four
128-row tiles {j, 8+j, 16+j, 24+j} of A and C (512 rows per core); B
(triu-masked) is replicated. All cores run the identical program; where the
program's k-range extends past a core's actual triangle the masked A columns
are zero, so the extra matmuls accumulate zeros and stay correct. This makes
the per-core instruction streams (and hence runtimes) identical by
construction - no load imbalance.

Per-core program: the transposed A shard (lhsT layout, [128, 32, 512]) stays
SBUF-resident; B is streamed tile-by-tile ([128, 512], each tile touched
exactly once); C accumulates in PSUM banks (up to 4 live), is copied out via
VectorE and DMAed to DRAM.

Loop structure: for each 512-wide column super-block s (8 of them), for each
k-tile <= 4s+3, load B[k, s-block] once and matmul it against the A tiles of
every owned row-slot t with 8t <= k, accumulating into psum[t].
"""

import os
import sys

for _p in ("/opt/trn_rl_repo", "/root/.axon_site/_ro/trn_rl_repo"):
    if _p not in sys.path:
        sys.path.insert(0, _p)

import numpy as np

N = 4096
P = 128
NCORES = 8
NSLOT = 4  # row-tiles per core
SW = 512  # n super-block width
NS = N // SW  # 8 supers
KT = N // P  # 32 k-tiles

# matmul input dtype: "f32" (exact, 4 cyc/row), "f32r" (tf32-like, 1 cyc/row
# at free dim >= 256), "bf16" (1 cyc/row, half the DMA traffic)
MM_DTYPE = os.environ.get("MM_DTYPE", "f32")

_cache = {}


def _build(dt_mode):
    import concourse.bacc as bacc
    import concourse.mybir as mybir
    import concourse.tile as tile

    D = {
        "f32": mybir.dt.float32,
        "f32r": mybir.dt.float32r,
        "bf16": mybir.dt.bfloat16,
    }[dt_mode]

    nc = bacc.Bacc(None, target_bir_lowering=False)
    AT = nc.dram_tensor("AT", [P, KT, NSLOT * P], D, kind="ExternalInput")
    # B packed per n-super: B_packed[s, p, ko, w] = triu(B)[128*ko + p, 512*s + w]
    # so a k-chunk load is per-partition contiguous (KCHUNK*512 elements).
    Bm = nc.dram_tensor("B", [NS, P, KT, SW], D, kind="ExternalInput")
    Cm = nc.dram_tensor("C", [NSLOT * P, N], mybir.dt.float32, kind="ExternalOutput")

    KCHUNK = 4
    b_bufs = 12 if dt_mode == "bf16" else 6

    with tile.TileContext(nc) as tc:
        with (
            tc.tile_pool(name="a", bufs=4) as apool,
            tc.tile_pool(name="b", bufs=b_bufs) as bpool,
            tc.tile_pool(name="o", bufs=4) as opool,
            tc.tile_pool(name="ps", bufs=8, space="PSUM") as pspool,
        ):
            # A shard resident in 4 independent tiles so early matmuls only
            # wait on the first chunk
            # A loads go on the Scalar engine's DMA queue so they stream in
            # parallel with the B chunks issued from the Sync queue
            a_tiles = []
            for g in range(4):
                ag = apool.tile([P, 8, NSLOT * P], D, tag=f"a{g}", name="ag")
                nc.scalar.dma_start(ag[:], AT[:, 8 * g : 8 * (g + 1), :])
                a_tiles.append(ag)

            for s in range(NS):
                kmax = 4 * s + 3
                nslots = kmax // 8 + 1
                psums = [
                    pspool.tile([P, SW], mybir.dt.float32, tag="ps", name="ps")
                    for _ in range(nslots)
                ]
                for kc in range(0, kmax + 1, KCHUNK):
                    cnt = min(KCHUNK, kmax + 1 - kc)
                    bt = bpool.tile([P, KCHUNK, SW], D, tag="b", name="bt")
                    nc.sync.dma_start(bt[:, :cnt, :], Bm[s, :, kc : kc + cnt, :])
                    for k in range(kc, kc + cnt):
                        # columns left of 128*(k - 4s) are k < n-tile regions
                        # where triu(B) is zero; skip them
                        w0 = max(0, P * (k - 4 * s))
                        for t in range(k // 8 + 1):
                            nc.tensor.matmul(
                                psums[t][:, w0:SW],
                                a_tiles[k // 8][:, k % 8, P * t : P * (t + 1)],
                                bt[:, k - kc, w0:SW],
                                start=(k == 8 * t),
                                stop=(k == kmax),
                            )
                for t in range(nslots):
                    ot = opool.tile([P, SW], mybir.dt.float32, tag="o", name="ot")
                    nc.vector.tensor_copy(ot[:], psums[t][:])
                    # C stores on the GpSimd queue: keeps the Sync queue free
                    # for B streaming
                    nc.gpsimd.dma_start(
                        Cm[P * t : P * (t + 1), SW * s : SW * (s + 1)], ot[:]
                    )
    nc.compile()
    return nc


def _get_nc():
    if MM_DTYPE not in _cache:
        _cache[MM_DTYPE] = _build(MM_DTYPE)
    return _cache[MM_DTYPE]


def _np_dtype():
    if MM_DTYPE == "bf16":
        import ml_dtypes

        return np.dtype(ml_dtypes.bfloat16)
    return np.dtype(np.float32)


def _make_in_maps(A, B):
    A = np.asarray(A, dtype=np.float32)
    B = np.asarray(B, dtype=np.float32)
    Au = np.triu(A)
    Bu = np.triu(B)

    npdt = _np_dtype()
    # pack: B_packed[s, p, ko, w] = Bu[128*ko + p, 512*s + w]
    Bu_c = np.ascontiguousarray(
        Bu.reshape(KT, P, NS, SW).transpose(2, 1, 0, 3)
    )
    if npdt != np.float32:
        Bu_c = Bu_c.astype(npdt)

    in_maps = []
    for j in range(NCORES):
        rows = np.concatenate(
            [
                np.arange(P * (NCORES * t + j), P * (NCORES * t + j) + P)
                for t in range(NSLOT)
            ]
        )
        A_loc = Au[rows, :]  # [512, 4096]
        # lhsT layout [p, ko, ml]: element = A_loc[ml, ko*128 + p]
        ATd = np.ascontiguousarray(
            A_loc.reshape(NSLOT * P, KT, P).transpose(2, 1, 0)
        )
        if npdt != np.float32:
            ATd = ATd.astype(npdt)
        in_maps.append({"AT": ATd, "B": Bu_c})
    return in_maps


def kernel(A, B):
    from concourse.bass_utils import run_bass_kernel_spmd

    in_maps = _make_in_maps(A, B)
    nc = _get_nc()
    res = run_bass_kernel_spmd(nc, in_maps, core_ids=list(range(NCORES)))

    C = np.zeros((N, N), dtype=np.float32)
    for j in range(NCORES):
        Cj = res.results[j]["C"]
        for t in range(NSLOT):
            m = NCORES * t + j
            C[P * m : P * (m + 1), :] = Cj[P * t : P * (t + 1), :]
    return C
